# revision 7
# baseline (speedup 1.0000x reference)
import sys

sys.path.insert(0, "/opt/trn_rl_repo")

import numpy as np

import concourse.bass as bass
from concourse import bacc
import concourse.mybir as mybir
from concourse.bass_utils import run_bass_kernel_spmd
from concourse.masks import make_identity
from concourse.tile import TileContext

F32 = mybir.dt.float32
AF = mybir.ActivationFunctionType
ALU = mybir.AluOpType

V, E, H = 128, 32, 128
B, S = 4, 512
NC = 8  # cores
# core i: batch b = i // 2, s-half = i % 2 (two 128-row s-blocks each)

_cache = {}


def _build_nc():
    """One SPMD program for all 8 cores. Per-core behavior differs only via
    input data (x row, selection matrices, causal mask)."""
    nc = bacc.Bacc()

    # ---- DRAM I/O ----
    xf = nc.dram_tensor("xf", [S], F32, kind="ExternalInput")  # token ids as f32
    iota = nc.dram_tensor("iota", [128, 1], F32, kind="ExternalInput")
    emb_d = nc.dram_tensor("emb", [V, E], F32, kind="ExternalInput")
    whhT_d = nc.dram_tensor("whhT", [H, 4 * H], F32, kind="ExternalInput")
    wihT_d = nc.dram_tensor("wihT", [E, 4 * H], F32, kind="ExternalInput")
    biasg_d = nc.dram_tensor("biasg", [H, 4], F32, kind="ExternalInput")
    wqT_d = nc.dram_tensor("wqT", [H, H], F32, kind="ExternalInput")
    bq_d = nc.dram_tensor("bq", [H, 1], F32, kind="ExternalInput")
    wkT_d = nc.dram_tensor("wkT", [H, H], F32, kind="ExternalInput")
    bk_d = nc.dram_tensor("bk", [H, 1], F32, kind="ExternalInput")
    v_d = nc.dram_tensor("v", [H, 1], F32, kind="ExternalInput")
    wf1T_d = nc.dram_tensor("wf1T", [H, V], F32, kind="ExternalInput")
    wf2T_d = nc.dram_tensor("wf2T", [H, V], F32, kind="ExternalInput")
    bf_d = nc.dram_tensor("bf", [V, 1], F32, kind="ExternalInput")
    sel_d = nc.dram_tensor("sel", [2, 4, 128, 128], F32, kind="ExternalInput")
    mask_d = nc.dram_tensor("mask", [2, 128, S], F32, kind="ExternalInput")

    olg_d = nc.dram_tensor("olg", [2, V, 128], F32, kind="ExternalOutput")
    ohc_d = nc.dram_tensor("ohc", [H, 2], F32, kind="ExternalOutput")

    with TileContext(nc) as tc:
        with (
            tc.tile_pool(name="singles", bufs=1) as singles,
            tc.tile_pool(name="state", bufs=3) as state,
            tc.tile_pool(name="gtmp", bufs=3) as gtmp,
            tc.tile_pool(name="th", bufs=3) as thp,
            tc.tile_pool(name="blk", bufs=2) as blkp,
            tc.tile_pool(name="ps_small", bufs=3, space="PSUM") as psS,
            tc.tile_pool(name="ps_big", bufs=2, space="PSUM") as psB,
            tc.tile_pool(name="ps_acc", bufs=1, space="PSUM") as psACC,
            tc.tile_pool(name="ps_sct", bufs=2, space="PSUM") as psSCT,
        ):
            # ---- load constants ----
            ident = singles.tile([128, 128], F32)
            make_identity(nc, ident)
            emb_sb = singles.tile([V, E], F32)
            nc.sync.dma_start(emb_sb, emb_d[:, :])
            whhT = singles.tile([H, 4 * H], F32)
            nc.sync.dma_start(whhT, whhT_d[:, :])
            wihT = singles.tile([E, 4 * H], F32)
            nc.sync.dma_start(wihT, wihT_d[:, :])
            biasg = singles.tile([H, 4], F32)
            nc.sync.dma_start(biasg, biasg_d[:, :])
            wqT = singles.tile([H, H], F32)
            nc.sync.dma_start(wqT, wqT_d[:, :])
            bq = singles.tile([H, 1], F32)
            nc.sync.dma_start(bq, bq_d[:, :])
            wkT = singles.tile([H, H], F32)
            nc.sync.dma_start(wkT, wkT_d[:, :])
            bk = singles.tile([H, 1], F32)
            nc.sync.dma_start(bk, bk_d[:, :])
            vcol = singles.tile([H, 1], F32)
            nc.sync.dma_start(vcol, v_d[:, :])
            wf1T = singles.tile([H, V], F32)
            nc.sync.dma_start(wf1T, wf1T_d[:, :])
            wf2T = singles.tile([H, V], F32)
            nc.sync.dma_start(wf2T, wf2T_d[:, :])
            bf = singles.tile([V, 1], F32)
            nc.sync.dma_start(bf, bf_d[:, :])
            iota_sb = singles.tile([128, 1], F32)
            nc.sync.dma_start(iota_sb, iota[:, :])
            sel_sb = singles.tile([128, 2, 4, 128], F32)
            nc.sync.dma_start(
                sel_sb,
                bass.AP(sel_d, 0, [[128, 128], [4 * 128 * 128, 2], [128 * 128, 4], [1, 128]]),
            )
            mask_sb = singles.tile([128, 2, S], F32)
            nc.sync.dma_start(
                mask_sb, bass.AP(mask_d, 0, [[S, 128], [128 * S, 2], [1, S]])
            )
            # x broadcast to all 128 partitions
            xb = singles.tile([128, S], F32)
            nc.sync.dma_start(xb, bass.AP(xf, 0, [[0, 128], [1, S]]))

            zcol = singles.tile([128, 1], F32)
            nc.vector.memset(zcol, 0.0)

            # PE operands must have a single wait-sem domain (ISA limit:
            # one sync wait per Matmult). Launder everything the PE reads
            # through DVE (vcol through ACT - th tiles are ACT-produced).
            def dve_twin(ap, nm):
                tw = singles.tile(list(ap.shape), F32, name=nm)
                nc.vector.tensor_copy(tw, ap)
                return tw

            ident = dve_twin(ident, "ident2")
            emb_sb = dve_twin(emb_sb, "emb2")
            whhT = dve_twin(whhT, "whhT2")
            wihT = dve_twin(wihT, "wihT2")
            wqT = dve_twin(wqT, "wqT2")
            wkT = dve_twin(wkT, "wkT2")
            wf1T = dve_twin(wf1T, "wf1T2")
            wf2T = dve_twin(wf2T, "wf2T2")
            sel2 = singles.tile([128, 2, 4, 128], F32, name="sel2")
            nc.vector.tensor_copy(sel2, sel_sb)
            sel_sb = sel2
            vcol2 = singles.tile([H, 1], F32, name="vcol2")
            nc.scalar.copy(vcol2, vcol)
            vcol = vcol2

            # ---- embedding via one-hot matmul ----
            onehot = singles.tile([V, S], F32)
            nc.vector.tensor_scalar(onehot, xb, iota_sb, None, ALU.is_equal)
            eT_ps = psB.tile([E, S], F32, tag="big")
            nc.tensor.matmul(eT_ps, emb_sb, onehot, start=True, stop=True)
            eT = singles.tile([E, S], F32)
            nc.vector.tensor_copy(eT, eT_ps)

            # ---- input-side gate preactivations P[:, t, g] = W_ih e_t + bias ----
            P_sb = singles.tile([128, S, 4], F32)
            for g in range(4):
                pg_ps = psB.tile([128, S], F32, tag="big")
                nc.tensor.matmul(
                    pg_ps, wihT[:, g * 128 : (g + 1) * 128], eT, start=True, stop=True
                )
                nc.vector.tensor_scalar(
                    P_sb[:, :, g], pg_ps, biasg[:, g : g + 1], None, ALU.add
                )

            # ---- LSTM (gate cols: 0=i 1=f 2=o 3=g) ----
            outT = [singles.tile([H, 128], F32, name=f"outT{c}") for c in range(4)]
            c_prev = zcol
            h_prev = zcol
            for t in range(S):
                g_ps = psS.tile([128, 4], F32, tag="gps")
                for g in range(4):
                    nc.tensor.matmul(
                        g_ps[:, g : g + 1],
                        whhT[:, g * 128 : (g + 1) * 128],
                        h_prev,
                        start=True,
                        stop=True,
                    )
                g_sb = gtmp.tile([128, 4], F32, tag="gsb")
                nc.vector.tensor_tensor(g_sb, g_ps, P_sb[:, t, :], ALU.add)
                sg = gtmp.tile([128, 4], F32, tag="sg")
                nc.scalar.activation(sg[:, 0:3], g_sb[:, 0:3], AF.Sigmoid)
                nc.scalar.activation(sg[:, 3:4], g_sb[:, 3:4], AF.Tanh)
                m2 = state.tile([128, 1], F32, tag="m2")
                nc.vector.tensor_scalar(m2, sg[:, 3:4], sg[:, 0:1], None, ALU.mult)
                c_new = state.tile([128, 1], F32, tag="c")
                nc.vector.scalar_tensor_tensor(
                    c_new, c_prev, sg[:, 1:2], m2, ALU.mult, ALU.add
                )
                thc = state.tile([128, 1], F32, tag="thc")
                nc.scalar.activation(thc, c_new, AF.Tanh)
                h_new = state.tile([128, 1], F32, tag="h", bufs=4)
                nc.vector.tensor_scalar(h_new, thc, sg[:, 2:3], None, ALU.mult)
                nc.vector.tensor_copy(outT[t // 128][:, t % 128 : t % 128 + 1], h_new)
                c_prev = c_new
                h_prev = h_new

            nc.sync.dma_start(ohc_d[:, 0:1], h_prev)
            nc.sync.dma_start(ohc_d[:, 1:2], c_prev)

            # ---- k^T over full sequence; out_nat chunks ----
            kT = singles.tile([H, S], F32)
            on_sb = [singles.tile([128, H], F32, name=f"on{c}") for c in range(4)]
            for c in range(4):
                kps = psB.tile([H, 128], F32, tag="big")
                nc.tensor.matmul(kps, wkT, outT[c], start=True, stop=True)
                nc.vector.tensor_scalar(
                    kT[:, c * 128 : (c + 1) * 128], kps, bk, None, ALU.add
                )
                tps = psB.tile([128, H], F32, tag="big")
                nc.tensor.transpose(tps, outT[c], ident)
                nc.vector.tensor_copy(on_sb[c], tps)

            # ---- attention per block ----
            for blk in range(2):
                ost_ps = psB.tile([H, 128], F32, tag="big")
                for c in range(4):
                    nc.tensor.matmul(
                        ost_ps,
                        on_sb[c],
                        sel_sb[:, blk, c, :],
                        start=(c == 0),
                        stop=(c == 3),
                    )
                ost = blkp.tile([H, 128], F32, tag="ost_sb")
                nc.vector.tensor_copy(ost, ost_ps)
                qs_ps = psB.tile([H, 128], F32, tag="big")
                nc.tensor.matmul(qs_ps, wqT, ost, start=True, stop=True)
                qsel = blkp.tile([H, 128], F32, tag="qsel")
                nc.vector.tensor_scalar(qsel, qs_ps, bq, None, ALU.add)

                # scores^T columns: SCT[:, tc*128+j] = th_j[:, tc-chunk].T @ v
                sct_ps = psSCT.tile([128, S], F32, tag="sct")
                for j in range(128):
                    th = thp.tile([H, S], F32, tag="th")
                    nc.scalar.activation(th, kT, AF.Tanh, bias=qsel[:, j : j + 1])
                    for c in range(4):
                        nc.tensor.matmul(
                            sct_ps[:, c * 128 + j : c * 128 + j + 1],
                            th[:, c * 128 : (c + 1) * 128],
                            vcol,
                            start=True,
                            stop=True,
                        )
                # gather scores to [j, t] with mask
                sc = blkp.tile([128, S], F32, tag="sc")
                for c in range(4):
                    stmp = blkp.tile([128, 128], F32, tag="stmp")
                    nc.vector.tensor_copy(stmp, sct_ps[:, c * 128 : (c + 1) * 128])
                    scps = psB.tile([128, 128], F32, tag="big")
                    nc.tensor.transpose(scps, stmp, ident)
                    nc.vector.tensor_tensor(
                        sc[:, c * 128 : (c + 1) * 128],
                        scps,
                        mask_sb[:, blk, c * 128 : (c + 1) * 128],
                        ALU.add,
                    )
                # softmax rows
                mx = blkp.tile([128, 1], F32, tag="mx")
                nc.vector.tensor_reduce(mx, sc, mybir.AxisListType.X, ALU.max)
                nmx = blkp.tile([128, 1], F32, tag="nmx")
                nc.vector.tensor_scalar(nmx, mx, -1.0, None, ALU.mult)
                ew = blkp.tile([128, S], F32, tag="ew")
                zs = blkp.tile([128, 1], F32, tag="zs")
                nc.scalar.activation(ew, sc, AF.Exp, bias=nmx, accum_out=zs)
                rz = blkp.tile([128, 1], F32, tag="rz")
                nc.vector.reciprocal(rz, zs)
                wn = blkp.tile([128, S], F32, tag="wn")
                nc.vector.tensor_scalar(wn, ew, rz, None, ALU.mult)
                # ctx^T = sum_t out_nat[t,h]^T . wn^T[t,j]
                ctx_ps = psACC.tile([H, 128], F32, tag="acc")
                for c in range(4):
                    wnt_s = blkp.tile([128, 128], F32, tag="wnts")
                    nc.vector.tensor_copy(wnt_s, wn[:, c * 128 : (c + 1) * 128])
                    wnt_ps = psB.tile([128, 128], F32, tag="big")
                    nc.tensor.transpose(wnt_ps, wnt_s, ident)
                    wnt = blkp.tile([128, 128], F32, tag="wnt")
                    nc.vector.tensor_copy(wnt, wnt_ps)
                    nc.tensor.matmul(
                        ctx_ps, on_sb[c], wnt, start=(c == 0), stop=(c == 3)
                    )
                ctxT = blkp.tile([H, 128], F32, tag="ctxT")
                nc.vector.tensor_copy(ctxT, ctx_ps)
                # logits^T = Wf1 @ outsel^T + Wf2 @ ctx^T + bf
                lg_ps = psB.tile([V, 128], F32, tag="big")
                nc.tensor.matmul(lg_ps, wf1T, ost, start=True, stop=False)
                nc.tensor.matmul(lg_ps, wf2T, ctxT, start=False, stop=True)
                lg = blkp.tile([V, 128], F32, tag="lg")
                nc.vector.tensor_scalar(lg, lg_ps, bf, None, ALU.add)
                nc.sync.dma_start(olg_d[blk, :, :], lg)

    nc.compile()
    return nc


def _prep_inputs(x, emb, W_ih, W_hh, b_ih, b_hh, Wq, bq, Wk, bk, v, Wf, bf):
    x = np.asarray(x)
    f = lambda a: np.ascontiguousarray(np.asarray(a), dtype=np.float32)
    # gate reorder: torch chunks [i,f,g,o] -> our cols [i,f,o,g]
    perm = [0, 1, 3, 2]

    def gatecat(W):  # [4H, X] -> [X, 4H] with gate-major 128-col chunks
        chunks = [W[g * H : (g + 1) * H] for g in perm]
        return np.concatenate([c.T for c in chunks], axis=1)

    whhT = f(gatecat(np.asarray(W_hh)))  # [H, 4H]
    wihT = f(gatecat(np.asarray(W_ih)))  # [E, 4H]
    bias = np.asarray(b_ih) + np.asarray(b_hh)
    biasg = f(np.stack([bias[g * H : (g + 1) * H] for g in perm], axis=1))  # [H,4]

    base = dict(
        iota=f(np.arange(128)[:, None]),
        emb=f(emb),
        whhT=whhT,
        wihT=wihT,
        biasg=biasg,
        wqT=f(np.asarray(Wq).T),
        bq=f(np.asarray(bq)[:, None]),
        wkT=f(np.asarray(Wk).T),
        bk=f(np.asarray(bk)[:, None]),
        v=f(np.asarray(v)[:, None]),
        wf1T=f(np.asarray(Wf)[:, :H].T),
        wf2T=f(np.asarray(Wf)[:, H:].T),
        bf=f(np.asarray(bf)[:, None]),
    )
    in_maps = []
    for core in range(NC):
        b = core // 2
        pair = core % 2
        sel = np.zeros((2, 4, 128, 128), np.float32)
        mask = np.zeros((2, 128, S), np.float32)
        for blk in range(2):
            cglob = 2 * pair + blk  # global s-chunk handled by this block
            sel[blk, cglob] = np.eye(128, dtype=np.float32)
            for j in range(128):
                s_glob = cglob * 128 + j
                mask[blk, j, s_glob + 1 :] = -1e30
        m = dict(base)
        m["xf"] = f(x[b])
        m["sel"] = sel
        m["mask"] = mask
        in_maps.append(m)
    return in_maps


def kernel(x, emb, W_ih, W_hh, b_ih, b_hh, Wq, bq, Wk, bk, v, Wf, bf):
    if "nc" not in _cache:
        _cache["nc"] = _build_nc()
    nc = _cache["nc"]
    in_maps = _prep_inputs(
        x, emb, W_ih, W_hh, b_ih, b_hh, Wq, bq, Wk, bk, v, Wf, bf
    )
    res = run_bass_kernel_spmd(nc, in_maps, list(range(NC)))
    results = res.results
    logits = np.zeros((B, S, V), np.float32)
    hT = np.zeros((B, H), np.float32)
    cT = np.zeros((B, H), np.float32)
    for core in range(NC):
        b = core // 2
        pair = core % 2
        olg = results[core]["olg"]  # [2, V, 128]
        for blk in range(2):
            cglob = 2 * pair + blk
            logits[b, cglob * 128 : (cglob + 1) * 128, :] = olg[blk].T
        if pair == 0:
            ohc = results[core]["ohc"]
            hT[b] = ohc[:, 0]
            cT[b] = ohc[:, 1]
    return logits, hT, cT
